# revision 17
# baseline (speedup 1.0000x reference)
"""Trainium2 Bass kernel for sliding-window GQA attention (nn_Attention_62294205661445).

Sharding: 8 cores = 4 batches x 2 head-groups. Each core computes one batch's
attention for 8 q-heads / 2 kv-heads and a partial output projection over its
512 columns of the H*HD dim; the host sums the two partials per batch.

Per-core pipeline (all matmul operands bf16, 1 cyc/row; fp32 accumulate):
  phase 1: QKV projection (xT tiles, contraction d=1152), RoPE on [s,e] layout
           with stride-2 APs, PE-transpose q/k to [e,s], cache writeout.
  phase 2: transposed scores ST[j,i] = K·Q^T per 512-col supertile, additive
           sliding-window mask patterns, exp on ScalarE (no max subtraction:
           |scores*0.125| < ~6 so fp32 exp is safe), PV via ones-row-augmented
           V (row 64 of the PV psum accumulates the softmax denominator),
           reciprocal + gpsimd partition_broadcast, normalize, then WO.
"""
import sys
import numpy as np
import ml_dtypes

for p in ('/opt/trn_rl_repo',):
    if p not in sys.path:
        sys.path.insert(0, p)

import concourse.bass as bass
import concourse.tile as tile
from concourse import bacc
from concourse import mybir
from concourse.bass_utils import run_bass_kernel_spmd
from concourse.masks import make_identity

B, S, D = 4, 2048, 1152
H, KVH, HD = 16, 4, 64
SW = 1792
NT = S // 128          # 16 s-tiles
NG = NT // 4           # 4 supertiles (512 queries each)
DC = D // 128          # 9 contraction chunks
F32 = mybir.dt.float32
F32R = mybir.dt.float32r
BF16 = mybir.dt.bfloat16

_CACHED = {}


def r(ap):
    """View an AP as float32r (fast fp32 matmul mode)."""
    return ap.bitcast(F32R)


def build_graph():
    nc = bacc.Bacc()
    xT = nc.declare_dram_parameter("xT", [D, S], BF16, isOutput=False)
    wqT = nc.declare_dram_parameter("wqT", [D, 512], BF16, isOutput=False)
    wkvT = nc.declare_dram_parameter("wkvT", [D, 256], BF16, isOutput=False)
    woT = nc.declare_dram_parameter("woT", [512, D], BF16, isOutput=False)
    cosr = nc.declare_dram_parameter("cosrep", [S, 256], BF16, isOutput=False)
    sinr = nc.declare_dram_parameter("sinrep", [S, 256], BF16, isOutput=False)
    mdiag = nc.declare_dram_parameter("mdiagT", [128, 128], F32, isOutput=False)
    mtail = nc.declare_dram_parameter("mtailT", [128, 128], F32, isOutput=False)
    out_e = nc.declare_dram_parameter("out", [S, D], F32, isOutput=True)
    ck_e = nc.declare_dram_parameter("ck", [SW, 2, HD], F32, isOutput=True)
    cv_e = nc.declare_dram_parameter("cv", [SW, 2, HD], F32, isOutput=True)

    with tile.TileContext(nc) as tc:
        with (
            tc.tile_pool(name="const", bufs=1) as constp,
            tc.tile_pool(name="w", bufs=1) as wp,
            tc.tile_pool(name="persist", bufs=1) as pers,
            tc.tile_pool(name="xt", bufs=3) as xtp,
            tc.tile_pool(name="rope", bufs=3) as ropep,
            tc.tile_pool(name="ptile", bufs=12) as ppool,
            tc.tile_pool(name="outb", bufs=2) as outp,
            tc.tile_pool(name="small", bufs=2) as smallp,
            tc.tile_pool(name="attn", bufs=2) as atp,
            tc.tile_pool(name="psA", bufs=2, space="PSUM") as psA,
            tc.tile_pool(name="psB", bufs=2, space="PSUM") as psB,
            tc.tile_pool(name="psC", bufs=2, space="PSUM") as psC,
        ):
            ident = constp.tile([128, 128], BF16)
            make_identity(nc, ident[:])
            md_sb = constp.tile([128, 128], F32, tag="md")
            nc.scalar.dma_start(md_sb[:], mdiag[:])
            mt_sb = constp.tile([128, 128], F32, tag="mt")
            nc.scalar.dma_start(mt_sb[:], mtail[:])

            # resident weights / tables
            wq_sb = wp.tile([128, DC, 512], BF16, tag="wq")
            nc.sync.dma_start(
                wq_sb[:], wqT[:].rearrange("(c p) e -> p c e", p=128))
            wkv_sb = wp.tile([128, DC, 256], BF16, tag="wkv")
            nc.sync.dma_start(
                wkv_sb[:], wkvT[:].rearrange("(c p) e -> p c e", p=128))
            wo_sb = wp.tile([128, 4, D], BF16, tag="wo")
            nc.gpsimd.dma_start(
                wo_sb[:], woT[:].rearrange("(c p) e -> p c e", p=128))
            cos_sb = wp.tile([128, NT, 256], BF16, tag="cos")
            nc.scalar.dma_start(
                cos_sb[:], cosr[:].rearrange("(t p) m -> p t m", p=128))
            sin_sb = wp.tile([128, NT, 256], BF16, tag="sin")
            nc.scalar.dma_start(
                sin_sb[:], sinr[:].rearrange("(t p) m -> p t m", p=128))

            # persistent activations
            qT_all = pers.tile([128, 4, S], BF16, tag="qT")     # [e-part, etile, s]
            kT_all = pers.tile([128, S], BF16, tag="kT")        # [e-part, s]
            vaug = pers.tile([128, NT, 130], BF16, tag="vaug")  # [j, t, 2*(64+1)]
            nc.gpsimd.memset(vaug[:, :, 64::65], 1.0)

            # ---------------- phase 1: projections + RoPE + transposes -------
            stash = {}
            for t in range(NT):
                xt = xtp.tile([128, DC, 128], BF16, tag="xt")
                nc.sync.dma_start(
                    xt[:], xT[:, t*128:(t+1)*128].rearrange(
                        "(c p) s -> p c s", p=128))
                q_ps = psA.tile([128, 512], F32, tag="A")
                kv_ps = psB.tile([128, 256], F32, tag="B")
                for c in range(DC):
                    nc.tensor.matmul(q_ps[:], xt[:, c, :], wq_sb[:, c, :],
                                     start=(c == 0), stop=(c == DC-1))
                for c in range(DC):
                    nc.tensor.matmul(kv_ps[:], xt[:, c, :], wkv_sb[:, c, :],
                                     start=(c == 0), stop=(c == DC-1))

                # RoPE: q
                qro = ropep.tile([128, 512], BF16, tag="qro")
                C = cos_sb[:, t, :]
                Sn = sin_sb[:, t, :]
                ta = ropep.tile([128, 256], F32, tag="ta")
                tb = ropep.tile([128, 256], F32, tag="tb")
                a, b = q_ps[:, 0:512:2], q_ps[:, 1:512:2]
                nc.vector.tensor_mul(ta[:], a, C)
                nc.vector.tensor_mul(tb[:], b, Sn)
                nc.vector.tensor_sub(qro[:, 0:512:2], ta[:], tb[:])
                nc.vector.tensor_mul(ta[:], a, Sn)
                nc.vector.tensor_mul(tb[:], b, C)
                nc.vector.tensor_add(qro[:, 1:512:2], ta[:], tb[:])
                # RoPE: k
                kro = ropep.tile([128, 128], BF16, tag="kro")
                ka, kb = kv_ps[:, 0:128:2], kv_ps[:, 1:128:2]
                C2, S2 = cos_sb[:, t, 0:64], sin_sb[:, t, 0:64]
                ka1 = ropep.tile([128, 64], F32, tag="ka1")
                kb1 = ropep.tile([128, 64], F32, tag="kb1")
                nc.vector.tensor_mul(ka1[:], ka, C2)
                nc.vector.tensor_mul(kb1[:], kb, S2)
                nc.vector.tensor_sub(kro[:, 0:128:2], ka1[:], kb1[:])
                nc.vector.tensor_mul(ka1[:], ka, S2)
                nc.vector.tensor_mul(kb1[:], kb, C2)
                nc.vector.tensor_add(kro[:, 1:128:2], ka1[:], kb1[:])

                # v into augmented layout (ScalarE copy, psum->sbuf)
                nc.scalar.copy(
                    vaug[:, t, :].rearrange("p (h x) -> p h x", h=2)[:, :, 0:64],
                    kv_ps[:, 128:256].rearrange("p (h x) -> p h x", h=2))

                # cache writeout for s >= 256
                if t >= 2:
                    s0 = t * 128
                    slot = s0 - SW if s0 >= SW else s0
                    nc.gpsimd.dma_start(
                        ck_e[slot:slot+128],
                        kro[:].rearrange("p (h x) -> p h x", h=2))
                    nc.gpsimd.dma_start(
                        cv_e[slot:slot+128],
                        vaug[:, t, :].rearrange("p (h x) -> p h x", h=2)[:, :, 0:64])

                # transposes to [e, s] layout (PE transpose, bf16)
                for cc in range(4):
                    tr = psC.tile([128, 128], BF16, tag="C")
                    nc.tensor.transpose(tr[:], qro[:, cc*128:(cc+1)*128], ident[:])
                    nc.vector.tensor_copy(qT_all[:, cc, t*128:(t+1)*128], tr[:])
                trk = psC.tile([128, 128], BF16, tag="C")
                nc.tensor.transpose(trk[:], kro[:], ident[:])
                nc.vector.tensor_copy(kT_all[:, t*128:(t+1)*128], trk[:])

            # ---------------- phase 2: attention + WO ----------------------
            for gi in range(NG):
                at_sb = atp.tile([128, 4, 512], BF16, tag="at")
                for hl in range(8):
                    # q heads are stored in order [0,4,1,5,2,6,3,7] (host-side
                    # weight permutation) so head hl sits at partition half
                    # hl//4 == its kv head — matmul operand bases then align.
                    h2 = hl // 4
                    qp0 = h2 * 64
                    qrow = hl % 4
                    kj0, kj1 = max(0, 4*gi - 14), 4*gi + 3
                    ptiles = {}
                    pv = psB.tile([65, 512], F32, tag="B")
                    kjps = list(range(kj0, kj1 + 1, 2))

                    def emit_pv(kjp):
                        for kj in (kjp, kjp + 1):
                            p_, o_ = ptiles[kj]
                            nc.tensor.matmul(
                                pv[:],
                                vaug[:, kj, h2*65:(h2+1)*65],
                                p_[:, o_:o_+512],
                                start=(kj == kj0), stop=(kj == kj1))

                    for kjp in kjps:
                        # two adjacent key blocks share one [128,1024] psum
                        # tile (2 banks) so exp runs as a single wide op
                        st = psA.tile([128, 1024], F32, tag="A")
                        p = ppool.tile([128, 1024], BF16, tag="p")
                        ranges = []
                        for half, kj in enumerate((kjp, kjp + 1)):
                            o = half * 512
                            dt_ = kj - 4*gi
                            t_start = max(0, dt_)
                            tail_t = dt_ + 14
                            t_end = min(3, tail_t)
                            W0, W1 = o + t_start*128, o + (t_end+1)*128
                            nc.tensor.matmul(
                                st[:, W0:W1],
                                kT_all[h2*64:(h2+1)*64, kj*128:(kj+1)*128],
                                qT_all[qp0:qp0+64, qrow,
                                       gi*512+W0-o:gi*512+W1-o],
                                start=True, stop=True)
                            if 0 <= dt_ <= 3:
                                sl = st[:, o+dt_*128:o+(dt_+1)*128]
                                nc.vector.tensor_add(sl, sl, md_sb[:])
                            if tail_t <= 3:
                                sl = st[:, o+tail_t*128:o+(tail_t+1)*128]
                                nc.vector.tensor_add(sl, sl, mt_sb[:])
                            if W0 > o:
                                nc.gpsimd.memset(p[:, o:W0], 0.0)
                            if W1 < o + 512:
                                nc.gpsimd.memset(p[:, W1:o+512], 0.0)
                            ranges.append((W0, W1))
                            ptiles[kj] = (p, o)
                        # fuse the two halves' exp only when their valid
                        # ranges abut (else the gap holds unwritten psum)
                        if ranges[0][1] == ranges[1][0]:
                            ranges = [(ranges[0][0], ranges[1][1])]
                        for W0, W1 in ranges:
                            nc.scalar.activation(
                                p[:, W0:W1], st[:, W0:W1],
                                mybir.ActivationFunctionType.Exp, scale=0.125)
                        # PV of the previous kj-pair lands here so the PE has
                        # independent work while this pair's exp runs
                        if kjp != kjps[0]:
                            emit_pv(kjp - 2)
                    emit_pv(kjps[-1])
                    rs = smallp.tile([1, 512], F32, tag="rs")
                    nc.vector.tensor_copy(rs[:], pv[64:65, :])
                    rr = smallp.tile([1, 512], F32, tag="rr")
                    nc.vector.reciprocal_approx_fast(rr[:], rs[:])
                    Rb = smallp.tile([64, 512], F32, tag="Rb")
                    nc.gpsimd.partition_broadcast(Rb[:], rr[:])
                    nc.vector.tensor_mul(
                        at_sb[qp0:qp0+64, qrow, :], pv[0:64, :], Rb[:])
                # WO: out[s, d] partial for this supertile
                for ss in range(4):
                    osb = outp.tile([128, D], F32, tag="osb")
                    for nn in range(3):
                        wo_ps = psC.tile([128, 384], F32, tag="C")
                        for c in range(4):
                            nc.tensor.matmul(
                                wo_ps[:],
                                at_sb[:, c, ss*128:(ss+1)*128],
                                wo_sb[:, c, nn*384:(nn+1)*384],
                                start=(c == 0), stop=(c == 3))
                        nc.vector.tensor_copy(osb[:, nn*384:(nn+1)*384], wo_ps[:])
                    nc.sync.dma_start(
                        out_e[gi*512+ss*128:gi*512+(ss+1)*128, :], osb[:])
    nc.finalize()
    return nc


def _prep_inputs(x, wq, wk, wv, wo, freqs_cos, freqs_sin, mask):
    cosrep = np.ascontiguousarray(
        np.tile(np.asarray(freqs_cos), (1, 8))).astype(ml_dtypes.bfloat16)
    sinrep = np.ascontiguousarray(
        np.tile(np.asarray(freqs_sin), (1, 8))).astype(ml_dtypes.bfloat16)
    mdiagT = np.ascontiguousarray(np.asarray(mask)[0:128, 0:128].T,
                                  dtype=np.float32)
    mtailT = np.ascontiguousarray(np.asarray(mask)[1792:1920, 0:128].T,
                                  dtype=np.float32)
    # head-block permutation: etile c holds local heads (c, c+4) so that each
    # q head's partition half matches its kv head's half in kT
    hperm = np.array([0, 4, 1, 5, 2, 6, 3, 7])
    eperm = (hperm[:, None] * 64 + np.arange(64)[None, :]).reshape(-1)
    in_maps = []
    for b in range(B):
        for g in range(2):
            wq_g = np.asarray(wq)[512*g:512*(g+1)][eperm]      # [512, 1152]
            wo_g = np.asarray(wo)[:, 512*g:512*(g+1)][:, eperm]  # [1152, 512]
            m = {
                "xT": np.ascontiguousarray(np.asarray(x)[b].T).astype(ml_dtypes.bfloat16),
                "wqT": np.ascontiguousarray(wq_g.T).astype(ml_dtypes.bfloat16),
                "wkvT": np.ascontiguousarray(np.concatenate(
                    [np.asarray(wk)[128*g:128*(g+1)],
                     np.asarray(wv)[128*g:128*(g+1)]], 0).T).astype(
                         ml_dtypes.bfloat16),
                "woT": np.ascontiguousarray(wo_g.T).astype(ml_dtypes.bfloat16),
                "cosrep": cosrep, "sinrep": sinrep,
                "mdiagT": mdiagT, "mtailT": mtailT,
            }
            in_maps.append(m)
    return in_maps


def kernel(x, wq, wk, wv, wo, freqs_cos, freqs_sin, mask, cache_k, cache_v,
           positions, _trace=False):
    if 'nc' not in _CACHED:
        _CACHED['nc'] = build_graph()
    nc = _CACHED['nc']
    in_maps = _prep_inputs(x, wq, wk, wv, wo, freqs_cos, freqs_sin, mask)
    res = run_bass_kernel_spmd(nc, in_maps, core_ids=list(range(8)),
                               trace=_trace)
    outs = res.results
    out = np.zeros((B, S, D), np.float32)
    ck = np.zeros((B, SW, KVH, HD), np.float32)
    cv = np.zeros((B, SW, KVH, HD), np.float32)
    for b in range(B):
        for g in range(2):
            c = b*2 + g
            out[b] += outs[c]["out"]
            ck[b, :, 2*g:2*g+2] = outs[c]["ck"]
            cv[b, :, 2*g:2*g+2] = outs[c]["cv"]
    if _trace:
        return (out, ck, cv), res
    return out, ck, cv


# revision 18
# speedup vs baseline: 1.4815x; 1.4815x over previous
"""Trainium2 Bass kernel for sliding-window GQA attention (nn_Attention_62294205661445).

Sharding: 8 cores = 4 batches x 2 head-groups. Each core computes one batch's
attention for 8 q-heads / 2 kv-heads and a partial output projection over its
512 columns of the H*HD dim; the host sums the two partials per batch.

Per-core pipeline (all matmul operands bf16, 1 cyc/row; fp32 accumulate):
  phase 1: QKV projection (xT tiles, contraction d=1152), RoPE on [s,e] layout
           with stride-2 APs, PE-transpose q/k to [e,s], cache writeout.
  phase 2: transposed scores ST[j,i] = K·Q^T per 512-col supertile, additive
           sliding-window mask patterns, exp on ScalarE (no max subtraction:
           |scores*0.125| < ~6 so fp32 exp is safe), PV via ones-row-augmented
           V (row 64 of the PV psum accumulates the softmax denominator),
           reciprocal + gpsimd partition_broadcast, normalize, then WO.
"""
import sys
import numpy as np
import ml_dtypes

for p in ('/opt/trn_rl_repo',):
    if p not in sys.path:
        sys.path.insert(0, p)

import concourse.bass as bass
import concourse.tile as tile
from concourse import bacc
from concourse import mybir
from concourse.bass_utils import run_bass_kernel_spmd
from concourse.masks import make_identity

B, S, D = 4, 2048, 1152
H, KVH, HD = 16, 4, 64
SW = 1792
NT = S // 128          # 16 s-tiles
NG = NT // 4           # 4 supertiles (512 queries each)
DC = D // 128          # 9 contraction chunks
F32 = mybir.dt.float32
F32R = mybir.dt.float32r
BF16 = mybir.dt.bfloat16

_CACHED = {}


def r(ap):
    """View an AP as float32r (fast fp32 matmul mode)."""
    return ap.bitcast(F32R)


def build_graph():
    nc = bacc.Bacc()
    xT = nc.declare_dram_parameter("xT", [D, S], BF16, isOutput=False)
    wqT = nc.declare_dram_parameter("wqT", [D, 512], BF16, isOutput=False)
    wkvT = nc.declare_dram_parameter("wkvT", [D, 256], BF16, isOutput=False)
    woT = nc.declare_dram_parameter("woT", [512, D], BF16, isOutput=False)
    cosr = nc.declare_dram_parameter("cosrep", [S, 256], BF16, isOutput=False)
    sinr = nc.declare_dram_parameter("sinrep", [S, 256], BF16, isOutput=False)
    mdiag = nc.declare_dram_parameter("mdiagT", [128, 512], F32, isOutput=False)
    mtail = nc.declare_dram_parameter("mtailT", [128, 512], F32, isOutput=False)
    out_e = nc.declare_dram_parameter("out", [S, D], F32, isOutput=True)
    ck_e = nc.declare_dram_parameter("ck", [SW, 2, HD], F32, isOutput=True)
    cv_e = nc.declare_dram_parameter("cv", [SW, 2, HD], F32, isOutput=True)

    with tile.TileContext(nc) as tc:
        with (
            tc.tile_pool(name="const", bufs=1) as constp,
            tc.tile_pool(name="w", bufs=1) as wp,
            tc.tile_pool(name="persist", bufs=1) as pers,
            tc.tile_pool(name="xt", bufs=3) as xtp,
            tc.tile_pool(name="rope", bufs=3) as ropep,
            tc.tile_pool(name="ptile", bufs=12) as ppool,
            tc.tile_pool(name="outb", bufs=2) as outp,
            tc.tile_pool(name="small", bufs=2) as smallp,
            tc.tile_pool(name="attn", bufs=2) as atp,
            tc.tile_pool(name="psA", bufs=2, space="PSUM") as psA,
            tc.tile_pool(name="psB", bufs=2, space="PSUM") as psB,
            tc.tile_pool(name="psC", bufs=2, space="PSUM") as psC,
        ):
            ident = constp.tile([128, 128], BF16)
            make_identity(nc, ident[:])
            md_sb = constp.tile([128, 512], F32, tag="md")
            nc.scalar.dma_start(md_sb[:], mdiag[:])
            mt_sb = constp.tile([128, 512], F32, tag="mt")
            nc.scalar.dma_start(mt_sb[:], mtail[:])

            # resident weights / tables
            wq_sb = wp.tile([128, DC, 512], BF16, tag="wq")
            nc.sync.dma_start(
                wq_sb[:], wqT[:].rearrange("(c p) e -> p c e", p=128))
            wkv_sb = wp.tile([128, DC, 256], BF16, tag="wkv")
            nc.sync.dma_start(
                wkv_sb[:], wkvT[:].rearrange("(c p) e -> p c e", p=128))
            wo_sb = wp.tile([128, 4, D], BF16, tag="wo")
            nc.gpsimd.dma_start(
                wo_sb[:], woT[:].rearrange("(c p) e -> p c e", p=128))
            cos_sb = wp.tile([128, NT, 256], BF16, tag="cos")
            nc.scalar.dma_start(
                cos_sb[:], cosr[:].rearrange("(t p) m -> p t m", p=128))
            sin_sb = wp.tile([128, NT, 256], BF16, tag="sin")
            nc.scalar.dma_start(
                sin_sb[:], sinr[:].rearrange("(t p) m -> p t m", p=128))

            # persistent activations
            qT_all = pers.tile([128, 4, S], BF16, tag="qT")     # [e-part, etile, s]
            kT_all = pers.tile([128, S], BF16, tag="kT")        # [e-part, s]
            vaug = pers.tile([128, NT, 130], BF16, tag="vaug")  # [j, t, 2*(64+1)]
            nc.gpsimd.memset(vaug[:, :, 64::65], 1.0)

            # ---------------- phase 1: projections + RoPE + transposes -------
            stash = {}
            for t in range(NT):
                xt = xtp.tile([128, DC, 128], BF16, tag="xt")
                nc.sync.dma_start(
                    xt[:], xT[:, t*128:(t+1)*128].rearrange(
                        "(c p) s -> p c s", p=128))
                q_ps = psA.tile([128, 512], F32, tag="A")
                kv_ps = psB.tile([128, 256], F32, tag="B")
                for c in range(DC):
                    nc.tensor.matmul(q_ps[:], xt[:, c, :], wq_sb[:, c, :],
                                     start=(c == 0), stop=(c == DC-1))
                for c in range(DC):
                    nc.tensor.matmul(kv_ps[:], xt[:, c, :], wkv_sb[:, c, :],
                                     start=(c == 0), stop=(c == DC-1))

                # RoPE: q
                qro = ropep.tile([128, 512], BF16, tag="qro")
                C = cos_sb[:, t, :]
                Sn = sin_sb[:, t, :]
                ta = ropep.tile([128, 256], F32, tag="ta")
                tb = ropep.tile([128, 256], F32, tag="tb")
                a, b = q_ps[:, 0:512:2], q_ps[:, 1:512:2]
                nc.vector.tensor_mul(ta[:], a, C)
                nc.vector.tensor_mul(tb[:], b, Sn)
                nc.vector.tensor_sub(qro[:, 0:512:2], ta[:], tb[:])
                nc.vector.tensor_mul(ta[:], a, Sn)
                nc.vector.tensor_mul(tb[:], b, C)
                nc.vector.tensor_add(qro[:, 1:512:2], ta[:], tb[:])
                # RoPE: k
                kro = ropep.tile([128, 128], BF16, tag="kro")
                ka, kb = kv_ps[:, 0:128:2], kv_ps[:, 1:128:2]
                C2, S2 = cos_sb[:, t, 0:64], sin_sb[:, t, 0:64]
                ka1 = ropep.tile([128, 64], F32, tag="ka1")
                kb1 = ropep.tile([128, 64], F32, tag="kb1")
                nc.vector.tensor_mul(ka1[:], ka, C2)
                nc.vector.tensor_mul(kb1[:], kb, S2)
                nc.vector.tensor_sub(kro[:, 0:128:2], ka1[:], kb1[:])
                nc.vector.tensor_mul(ka1[:], ka, S2)
                nc.vector.tensor_mul(kb1[:], kb, C2)
                nc.vector.tensor_add(kro[:, 1:128:2], ka1[:], kb1[:])

                # v into augmented layout (ScalarE copy, psum->sbuf)
                nc.scalar.copy(
                    vaug[:, t, :].rearrange("p (h x) -> p h x", h=2)[:, :, 0:64],
                    kv_ps[:, 128:256].rearrange("p (h x) -> p h x", h=2))

                # cache writeout for s >= 256
                if t >= 2:
                    s0 = t * 128
                    slot = s0 - SW if s0 >= SW else s0
                    nc.gpsimd.dma_start(
                        ck_e[slot:slot+128],
                        kro[:].rearrange("p (h x) -> p h x", h=2))
                    nc.gpsimd.dma_start(
                        cv_e[slot:slot+128],
                        vaug[:, t, :].rearrange("p (h x) -> p h x", h=2)[:, :, 0:64])

                # transposes to [e, s] layout (PE transpose, bf16)
                for cc in range(4):
                    tr = psC.tile([128, 128], BF16, tag="C")
                    nc.tensor.transpose(tr[:], qro[:, cc*128:(cc+1)*128], ident[:])
                    nc.vector.tensor_copy(qT_all[:, cc, t*128:(t+1)*128], tr[:])
                trk = psC.tile([128, 128], BF16, tag="C")
                nc.tensor.transpose(trk[:], kro[:], ident[:])
                nc.vector.tensor_copy(kT_all[:, t*128:(t+1)*128], trk[:])

            # ---------------- phase 2: attention + WO ----------------------
            # GQA-packed: one ST matmul computes scores of ALL 4 q-heads of a
            # kv group for one (q-tile, key-block): rhs = qT[64, 4 heads, 128]
            # -> st[j, 4*128]. One PV matmul serves all 4 heads (shared V).
            for gi in range(NG):
                at_sb = atp.tile([128, 4, 512], BF16, tag="at")
                for ql in range(4):
                    qi = gi * 4 + ql
                    for h2 in range(2):
                        kjs = list(range(max(0, qi - 14), qi + 1))
                        ptiles = {}
                        # ST + masks + exp, two kj per psum tile
                        for idx in range(0, len(kjs), 2):
                            pair = kjs[idx:idx+2]
                            st = psA.tile([128, 1024], F32, tag="A")
                            p = ppool.tile([128, 1024], BF16, tag="p")
                            for half, kj in enumerate(pair):
                                o = half * 512
                                nc.tensor.matmul(
                                    st[:, o:o+512],
                                    kT_all[h2*64:(h2+1)*64,
                                           kj*128:(kj+1)*128],
                                    qT_all[h2*64:h2*64+64, 0:4,
                                           qi*128:(qi+1)*128],
                                    start=True, stop=True)
                                if kj == qi:
                                    sl = st[:, o:o+512]
                                    nc.vector.tensor_add(sl, sl, md_sb[:])
                                elif kj == qi - 14:
                                    sl = st[:, o:o+512]
                                    nc.vector.tensor_add(sl, sl, mt_sb[:])
                                ptiles[kj] = (p, o)
                            nc.scalar.activation(
                                p[:, 0:512*len(pair)], st[:, 0:512*len(pair)],
                                mybir.ActivationFunctionType.Exp, scale=0.125)
                        # PV accumulation over kj (shared V for the 4 heads,
                        # ones row 64 collects the softmax denominators)
                        pv = psB.tile([65, 512], F32, tag="B")
                        for kj in kjs:
                            p_, o_ = ptiles[kj]
                            nc.tensor.matmul(
                                pv[:],
                                vaug[:, kj, h2*65:(h2+1)*65],
                                p_[:, o_:o_+512],
                                start=(kj == kjs[0]), stop=(kj == kjs[-1]))
                        rs = smallp.tile([1, 512], F32, tag="rs")
                        nc.vector.tensor_copy(rs[:], pv[64:65, :])
                        rr = smallp.tile([1, 512], F32, tag="rr")
                        nc.vector.reciprocal_approx_fast(rr[:], rs[:])
                        Rb = smallp.tile([64, 512], F32, tag="Rb")
                        nc.gpsimd.partition_broadcast(Rb[:], rr[:])
                        nc.vector.tensor_mul(
                            at_sb[h2*64:h2*64+64, :, ql*128:(ql+1)*128],
                            pv[0:64, :].rearrange("p (h x) -> p h x", h=4),
                            Rb[:].rearrange("p (h x) -> p h x", h=4))
                # WO: out[s, d] partial for this supertile
                for ss in range(4):
                    osb = outp.tile([128, D], F32, tag="osb")
                    for nn in range(3):
                        wo_ps = psC.tile([128, 384], F32, tag="C")
                        for c in range(4):
                            nc.tensor.matmul(
                                wo_ps[:],
                                at_sb[:, c, ss*128:(ss+1)*128],
                                wo_sb[:, c, nn*384:(nn+1)*384],
                                start=(c == 0), stop=(c == 3))
                        nc.vector.tensor_copy(osb[:, nn*384:(nn+1)*384], wo_ps[:])
                    nc.sync.dma_start(
                        out_e[gi*512+ss*128:gi*512+(ss+1)*128, :], osb[:])
    nc.finalize()
    return nc


def _prep_inputs(x, wq, wk, wv, wo, freqs_cos, freqs_sin, mask):
    cosrep = np.ascontiguousarray(
        np.tile(np.asarray(freqs_cos), (1, 8))).astype(ml_dtypes.bfloat16)
    sinrep = np.ascontiguousarray(
        np.tile(np.asarray(freqs_sin), (1, 8))).astype(ml_dtypes.bfloat16)
    mdiagT = np.ascontiguousarray(
        np.tile(np.asarray(mask)[0:128, 0:128].T, (1, 4)), dtype=np.float32)
    mtailT = np.ascontiguousarray(
        np.tile(np.asarray(mask)[1792:1920, 0:128].T, (1, 4)), dtype=np.float32)
    # head-block permutation: etile c holds local heads (c, c+4) so that each
    # q head's partition half matches its kv head's half in kT
    hperm = np.array([0, 4, 1, 5, 2, 6, 3, 7])
    eperm = (hperm[:, None] * 64 + np.arange(64)[None, :]).reshape(-1)
    in_maps = []
    for b in range(B):
        for g in range(2):
            wq_g = np.asarray(wq)[512*g:512*(g+1)][eperm]      # [512, 1152]
            wo_g = np.asarray(wo)[:, 512*g:512*(g+1)][:, eperm]  # [1152, 512]
            m = {
                "xT": np.ascontiguousarray(np.asarray(x)[b].T).astype(ml_dtypes.bfloat16),
                "wqT": np.ascontiguousarray(wq_g.T).astype(ml_dtypes.bfloat16),
                "wkvT": np.ascontiguousarray(np.concatenate(
                    [np.asarray(wk)[128*g:128*(g+1)],
                     np.asarray(wv)[128*g:128*(g+1)]], 0).T).astype(
                         ml_dtypes.bfloat16),
                "woT": np.ascontiguousarray(wo_g.T).astype(ml_dtypes.bfloat16),
                "cosrep": cosrep, "sinrep": sinrep,
                "mdiagT": mdiagT, "mtailT": mtailT,
            }
            in_maps.append(m)
    return in_maps


def kernel(x, wq, wk, wv, wo, freqs_cos, freqs_sin, mask, cache_k, cache_v,
           positions, _trace=False):
    if 'nc' not in _CACHED:
        _CACHED['nc'] = build_graph()
    nc = _CACHED['nc']
    in_maps = _prep_inputs(x, wq, wk, wv, wo, freqs_cos, freqs_sin, mask)
    res = run_bass_kernel_spmd(nc, in_maps, core_ids=list(range(8)),
                               trace=_trace)
    outs = res.results
    out = np.zeros((B, S, D), np.float32)
    ck = np.zeros((B, SW, KVH, HD), np.float32)
    cv = np.zeros((B, SW, KVH, HD), np.float32)
    for b in range(B):
        for g in range(2):
            c = b*2 + g
            out[b] += outs[c]["out"]
            ck[b, :, 2*g:2*g+2] = outs[c]["ck"]
            cv[b, :, 2*g:2*g+2] = outs[c]["cv"]
    if _trace:
        return (out, ck, cv), res
    return out, ck, cv
